# revision 7
# baseline (speedup 1.0000x reference)
"""Trainium2 Bass kernel for nn_CausalSelfAttention (GQA + RoPE + qk-RMSNorm).

Strategy (Megatron-style head parallelism over 8 NeuronCores):
  - Each core owns 2 of the 16 q heads and the matching 1 of 8 kv heads.
  - Per core: QKV projection for its 512 rows of w_attn, RoPE + qk RMS norm,
    causal flash-style attention for its (2 q heads x 2 batches), and a
    partial output projection through its 256 columns of w_proj.
  - Host sums the 8 partial outputs (no on-device collectives).

Schedule (v1):
  - All PSUM pools live at root scope (8 banks total) so attention(b0) can
    overlap QKV(b1), and out-proj(b0) can overlap attention(b1).
  - Group 0 runs its QKV matmuls m-inner with per-k DMA chunking so the PE
    starts ~1us into the kernel instead of waiting for the full x0/wq load.
  - RoPE runs per (head-tile, token-group) instead of per batch.
  - Softmax denominator: full k-blocks are pair-summed on GpSimd and
    quad-summed on DVE (bf16), so the PE's ones-matmul only sees ~1/4 of
    the columns; diagonal blocks go straight into the PE accumulation on
    their valid subranges.  Diagonal blocks skip the memset: exp writes the
    valid subrange, the 128-wide triangle gets a mask multiply, and all
    consumers read only the valid subrange.

Matmuls run in bf16 with fp32 PSUM accumulation; softmax statistics fp32.
Self-contained: hardcodes all shapes from the problem spec.
"""

import math
import numpy as np
import ml_dtypes
from contextlib import ExitStack

# ---- problem constants (hardcoded per spec) ----
B, T, C = 2, 2048, 2048
N_HEAD, N_KV_HEAD, HD = 16, 8, 128
KV_DIM = N_KV_HEAD * HD
EPS = 1.1920929e-07
N_CORES = 8
QH_PER_CORE = N_HEAD // N_CORES          # 2
TOK = B * T                              # 4096
P = 128
TG = 512                                 # token group (matmul N)
NT = TOK // TG                           # 8 token groups
KT = C // P                              # 16 contraction tiles
NGB = T // TG                            # 4 q groups per batch
NJB = T // P                             # 16 k tiles per batch
SCALE = 1.0 / math.sqrt(HD)

BF16 = ml_dtypes.bfloat16

_CACHE = {}


# --------------------------------------------------------------------------
# device program
# --------------------------------------------------------------------------

def _emit(tc, out_ap, t_in):
    import concourse.bass as bass  # noqa: F401
    import concourse.mybir as mybir

    f32 = mybir.dt.float32
    bf16 = mybir.dt.bfloat16
    AF = mybir.ActivationFunctionType
    nc = tc.nc

    x_d = t_in["x_sw"]
    wq_d = t_in["wq_sw"]
    wp_d = t_in["wp_sw"]
    cs_d = t_in["cs_sw"]
    mask_d = t_in["mask_sw"]
    eye_d = t_in["eye_sw"]

    with ExitStack() as root:
        # ---------------- PSUM pools (8 banks total, root scope) ----------
        mm_ps = root.enter_context(tc.tile_pool(name="mmps", bufs=2, space="PSUM"))
        ssq_ps = root.enter_context(tc.tile_pool(name="ssqps", bufs=1, space="PSUM"))
        s_ps = root.enter_context(tc.tile_pool(name="sps", bufs=2, space="PSUM"))
        y_ps = root.enter_context(tc.tile_pool(name="yps", bufs=2, space="PSUM"))
        d_ps = root.enter_context(tc.tile_pool(name="dps", bufs=1, space="PSUM"))

        # ---------------- SBUF pools --------------------------------------
        const = root.enter_context(tc.tile_pool(name="const", bufs=1))
        xin = root.enter_context(tc.tile_pool(name="xin", bufs=2))
        big = root.enter_context(tc.tile_pool(name="big", bufs=1))
        sqp = root.enter_context(tc.tile_pool(name="sq", bufs=2))
        sq2p = root.enter_context(tc.tile_pool(name="sq2", bufs=2))
        srp = root.enter_context(tc.tile_pool(name="sr", bufs=2))
        ropet = root.enter_context(tc.tile_pool(name="ropet", bufs=4))
        ptp = root.enter_context(tc.tile_pool(name="pt", bufs=18))
        prp = root.enter_context(tc.tile_pool(name="pr", bufs=6))
        qdp = root.enter_context(tc.tile_pool(name="qd", bufs=3))
        denp = root.enter_context(tc.tile_pool(name="den", bufs=2))
        ostgp = root.enter_context(tc.tile_pool(name="ostg", bufs=3))

        # ---------------- constant loads ----------------------------------
        # group 0 data interleaved per k-tile so the first matmul only waits
        # on ~256KB; supply rate then matches the PE's m-inner consume rate.
        wq_sb = const.tile([P, KT, TG], bf16)
        x0_sb = const.tile([P, KT, TG], bf16, tag="x0")
        for k in range(KT):
            nc.sync.dma_start(out=wq_sb[:, k], in_=wq_d[:, k])
            nc.sync.dma_start(out=x0_sb[:, k], in_=x_d[:, 0, k])
        cs_sb = const.tile([P, 2, T], bf16)
        nc.sync.dma_start(out=cs_sb[:], in_=cs_d)
        mask_sb = const.tile([P, P], bf16)
        nc.sync.dma_start(out=mask_sb[:], in_=mask_d)
        eye_sb = const.tile([P, P], bf16)
        nc.sync.dma_start(out=eye_sb[:], in_=eye_d)
        wp_sb = const.tile([P, QH_PER_CORE, C], bf16)
        nc.sync.dma_start(out=wp_sb[:], in_=wp_d)
        eps_sb = const.tile([P, 1], f32)
        nc.vector.memset(eps_sb[:], EPS)
        onesm_sb = const.tile([P, P], bf16)
        nc.vector.memset(onesm_sb[:], 1.0)

        # post-rope, post-norm q (2 heads) and k, in [d, tok] layout
        qn = [big.tile([P, TOK], bf16, name=f"qn{m}", tag=f"qn{m}") for m in range(3)]
        v_sb = big.tile([P, TOK], bf16, tag="v")
        vT_sb = big.tile([P, 2 * NJB, P], bf16, tag="vT")   # [ktok, (b,j), d]
        yT = [big.tile([P, TOK], bf16, name=f"yT{h}", tag=f"yT{h}") for h in range(QH_PER_CORE)]

        # ------- stage 1 helpers ------------------------------------------
        def qkv_consume(m, n, ps):
            """RMS-norm (m<3) or v-copy (m==3) of one QKV psum tile."""
            nsl = slice(n * TG, (n + 1) * TG)
            if m == 3:
                nc.vector.tensor_copy(v_sb[:, nsl], ps[:])
                return
            # cast to bf16 early (releases the PSUM bank), square on DVE so
            # the ACT engine only ever evaluates {Ln, Exp, Copy} — one table
            # set — and never thrashes table loads in the overlapped schedule.
            sq = sqp.tile([P, TG], bf16)
            nc.vector.tensor_copy(sq[:], ps[:])
            sq2 = sq2p.tile([P, TG], bf16)
            nc.vector.tensor_mul(sq2[:], sq[:], sq[:])
            ssqb = ssq_ps.tile([P, TG], f32)
            nc.tensor.matmul(ssqb[:], onesm_sb[:], sq2[:], start=True, stop=True)
            # rsqrt(ms + eps) = exp(-0.5 * ln(ms + eps))
            srb = srp.tile([P, TG], f32)
            nc.scalar.activation(srb[:], ssqb[:], AF.Ln, bias=eps_sb[:], scale=1.0 / HD)
            nc.scalar.activation(srb[:], srb[:], AF.Exp, scale=-0.5)
            nc.vector.tensor_mul(qn[m][:, nsl], sq[:], srb[:])

        def rope_group(m, n):
            """RoPE for one [128, TG] token-group of head-tile m (in place)."""
            nsl = slice(n * TG, (n + 1) * TG)
            csl = slice((n % NGB) * TG, (n % NGB + 1) * TG)
            t1 = ropet.tile([P, TG], bf16, tag="t1")
            xsw = ropet.tile([P, TG], bf16, tag="xsw")
            nc.gpsimd.dma_start(out=xsw[0:64, :], in_=qn[m][64:128, nsl])
            nc.gpsimd.dma_start(out=xsw[64:128, :], in_=qn[m][0:64, nsl])
            nc.vector.tensor_mul(t1[:], qn[m][:, nsl], cs_sb[:, 0, csl])
            # t1 = [x1*c ; x2*c]; xsw*s2n = [x2*s ; -x1*s]
            nc.vector.tensor_mul(xsw[:], xsw[:], cs_sb[:, 1, csl])
            nc.vector.tensor_add(qn[m][:, nsl], t1[:], xsw[:])

        def v_transpose_group(n):
            for blk in range(4 * n, 4 * n + 4):
                tp = mm_ps.tile([P, P], bf16, tag="mm")
                nc.tensor.transpose(tp[:], v_sb[:, blk * P:(blk + 1) * P], eye_sb[:])
                nc.vector.tensor_copy(vT_sb[:, blk], tp[:])

        def stage1_group(n, xb):
            for m in range(4):
                ps = mm_ps.tile([P, TG], f32, tag="mm")
                for k in range(KT):
                    nc.tensor.matmul(
                        ps[:], wq_sb[:, k, m * P:(m + 1) * P], xb[:, k],
                        start=(k == 0), stop=(k == KT - 1),
                    )
                qkv_consume(m, n, ps)
                if m < 3:
                    rope_group(m, n)
            v_transpose_group(n)

        # ------- stage 1, group 0: m-inner so the PE starts on k0 ---------
        ps0 = [mm_ps.tile([P, TG], f32, tag="mm", name="ps0a"),
               mm_ps.tile([P, TG], f32, tag="mm", name="ps0b"),
               y_ps.tile([P, TG], f32, tag="y", name="ps0c"),
               d_ps.tile([P, TG], f32, tag="d", name="ps0d")]
        for k in range(KT):
            for m in range(4):
                nc.tensor.matmul(
                    ps0[m][:], wq_sb[:, k, m * P:(m + 1) * P], x0_sb[:, k],
                    start=(k == 0), stop=(k == KT - 1),
                )
        for m in range(4):
            qkv_consume(m, 0, ps0[m])
            if m < 3:
                rope_group(m, 0)
        v_transpose_group(0)

        # ------- stage 1, groups 1..3 (batch 0) ---------------------------
        def stage1_rest(groups):
            for n in groups:
                xb = xin.tile([P, KT, TG], bf16)
                nc.sync.dma_start(out=xb[:, 0:8, :], in_=x_d[:, n, 0:8, :])
                nc.sync.dma_start(out=xb[:, 8:16, :], in_=x_d[:, n, 8:16, :])
                stage1_group(n, xb)

        stage1_rest(range(1, NGB))

        # ------- attention for one batch ----------------------------------
        def attention_batch(b):
            bT = b * T
            for qh in range(QH_PER_CORE):
                q_t, k_t = qn[qh], qn[2]
                for g in range(NGB):
                    qsl = slice(bT + g * TG, bT + (g + 1) * TG)
                    jmax = 4 * g + 3
                    pts = []
                    for j in range(jmax + 1):
                        off = (j - 4 * g) * P if j >= 4 * g else 0
                        sp = s_ps.tile([P, TG], f32)
                        nc.tensor.matmul(
                            sp[:, off:],
                            k_t[:, bT + j * P: bT + (j + 1) * P],
                            q_t[:, qsl][:, off:],
                            start=True, stop=True,
                        )
                        pt = ptp.tile([P, TG], bf16)
                        nc.scalar.activation(pt[:, off:], sp[:, off:], AF.Exp,
                                             scale=SCALE)
                        if j >= 4 * g:
                            # triangle mask on the diagonal 128 columns
                            nc.vector.tensor_mul(
                                pt[:, off:off + P], pt[:, off:off + P],
                                mask_sb[:, 0:P],
                            )
                        pts.append((pt, off))
                    # AV accumulation (diagonal blocks on their subranges)
                    yp = y_ps.tile([P, TG], f32, tag="y")
                    for idx, (pt, off) in enumerate(pts):
                        nc.tensor.matmul(
                            yp[:, off:], vT_sb[:, b * NJB + idx], pt[:, off:],
                            start=(idx == 0), stop=(idx == jmax),
                        )
                    # denominator: pair (gpsimd) + quad (DVE) tree over the
                    # full blocks, then a short ones-matmul accumulation.
                    nfull = 4 * g
                    pairs = []
                    for i in range(nfull // 2):
                        pr = prp.tile([P, TG], bf16)
                        nc.gpsimd.tensor_add(pr[:], pts[2 * i][0][:],
                                             pts[2 * i + 1][0][:])
                        pairs.append(pr)
                    quads = []
                    for i in range(nfull // 4):
                        qd = qdp.tile([P, TG], bf16)
                        nc.vector.tensor_add(qd[:], pairs[2 * i][:],
                                             pairs[2 * i + 1][:])
                        quads.append(qd)
                    terms = [(qd[:], 0) for qd in quads] + \
                            [(pt[:], off) for (pt, off) in pts[nfull:]]
                    dp = d_ps.tile([P, TG], f32, tag="d")
                    last = len(terms) - 1
                    for idx, (tsb, off) in enumerate(terms):
                        nc.tensor.matmul(
                            dp[:, off:], onesm_sb[:], tsb[:, off:],
                            start=(idx == 0), stop=(idx == last),
                        )
                    den = denp.tile([P, TG], f32)
                    nc.vector.reciprocal_approx_fast(den[:], dp[:])
                    nc.vector.tensor_mul(yT[qh][:, qsl], yp[:], den[:])

        # ------- output projection for one batch --------------------------
        def outproj_batch(b):
            for tt in range(b * NJB, (b + 1) * NJB):
                ost = ostgp.tile([P, C], bf16)
                for og in range(C // TG):
                    op = mm_ps.tile([P, TG], f32, tag="mm")
                    nc.tensor.matmul(
                        op[:], yT[0][:, tt * P:(tt + 1) * P],
                        wp_sb[:, 0, og * TG:(og + 1) * TG],
                        start=True, stop=False,
                    )
                    nc.tensor.matmul(
                        op[:], yT[1][:, tt * P:(tt + 1) * P],
                        wp_sb[:, 1, og * TG:(og + 1) * TG],
                        start=False, stop=True,
                    )
                    if og % 2 == 0:
                        nc.vector.tensor_copy(ost[:, og * TG:(og + 1) * TG], op[:])
                    else:
                        nc.scalar.copy(ost[:, og * TG:(og + 1) * TG], op[:])
                nc.sync.dma_start(out=out_ap[tt * P:(tt + 1) * P, :], in_=ost[:])

        # high priority: attention(b0); backfill: stage1(b1); then
        # attention(b1) with out-proj(b0) as backfill; then out-proj(b1).
        attention_batch(0)
        stage1_rest(range(NGB, NT))
        attention_batch(1)
        outproj_batch(0)
        outproj_batch(1)


def build_nc():
    """Build and compile the (single, shared across cores) Bass program."""
    if "nc" in _CACHE:
        return _CACHE["nc"]
    import concourse.mybir as mybir
    import concourse.tile as tile
    from concourse import bacc

    f32 = mybir.dt.float32  # noqa: F841
    bf16 = mybir.dt.bfloat16

    nc = bacc.Bacc("TRN2", target_bir_lowering=False, debug=False)
    shapes = {
        "x_sw": ((P, NT, KT, TG), bf16),
        "wq_sw": ((P, KT, TG), bf16),
        "wp_sw": ((P, QH_PER_CORE, C), bf16),
        "cs_sw": ((P, 2, T), bf16),
        "mask_sw": ((P, P), bf16),
        "eye_sw": ((P, P), bf16),
    }
    t_in = {
        name: nc.dram_tensor(name, shape, dt, kind="ExternalInput").ap()
        for name, (shape, dt) in shapes.items()
    }
    out_ap = nc.dram_tensor("out", (TOK, C), bf16, kind="ExternalOutput").ap()

    with tile.TileContext(nc) as tc:
        _emit(tc, out_ap, t_in)
    nc.compile()
    _CACHE["nc"] = nc
    return nc


# --------------------------------------------------------------------------
# host-side data preparation
# --------------------------------------------------------------------------

def _swizzle_ktiles(a2d):
    """[R*128, F] -> [128, R, F] picking partition-within-tile as leading."""
    r128, f = a2d.shape
    r = r128 // P
    return np.ascontiguousarray(a2d.reshape(r, P, f).transpose(1, 0, 2))


def host_prep(x, w_attn, w_proj, cos, sin):
    x = np.asarray(x, np.float32)
    w_attn = np.asarray(w_attn, np.float32)
    w_proj = np.asarray(w_proj, np.float32)
    cos = np.asarray(cos, np.float32).reshape(T, HD // 2)
    sin = np.asarray(sin, np.float32).reshape(T, HD // 2)

    # x: (B,T,C) -> xT (C, TOK) -> [128, n, k, t]
    xT = x.reshape(TOK, C).T                        # (C, TOK)
    x_sw = (
        xT.reshape(KT, P, NT, TG).transpose(1, 2, 0, 3)  # (P, n, k, t)
    )
    x_sw = np.ascontiguousarray(x_sw).astype(BF16)

    # cos/sin duplicated across both 64-partition halves: [128, 2, T]
    c2 = np.concatenate([cos.T, cos.T], axis=0)     # (128, T)
    s2 = np.concatenate([sin.T, -sin.T], axis=0)    # sign-folded for rope add
    cs_sw = np.stack([c2, s2], axis=1).astype(BF16)  # (128, 2, T)

    # causal triangle mask for the diagonal 128 columns: keep col >= row
    col = np.arange(P)[None, :]
    row = np.arange(P)[:, None]
    mask_sw = (col >= row).astype(BF16)              # (128, 128)

    eye_sw = np.eye(P, dtype=np.float32).astype(BF16)

    in_maps = []
    for c in range(N_CORES):
        qrows = w_attn[QH_PER_CORE * HD * c: QH_PER_CORE * HD * (c + 1)]
        krows = w_attn[C + HD * c: C + HD * (c + 1)]
        vrows = w_attn[C + KV_DIM + HD * c: C + KV_DIM + HD * (c + 1)]
        w_sel = np.concatenate([qrows, krows, vrows], axis=0)   # (512, C)
        wq_sw = _swizzle_ktiles(w_sel.T).astype(BF16)           # (128, 16, 512)

        wp_sel = w_proj[:, QH_PER_CORE * HD * c: QH_PER_CORE * HD * (c + 1)]
        wp_sw = _swizzle_ktiles(np.ascontiguousarray(wp_sel.T)).astype(BF16)

        in_maps.append({
            "x_sw": x_sw,
            "wq_sw": np.ascontiguousarray(wq_sw.reshape(P, KT, TG)),
            "wp_sw": np.ascontiguousarray(wp_sw.reshape(P, QH_PER_CORE, C)),
            "cs_sw": cs_sw,
            "mask_sw": mask_sw,
            "eye_sw": eye_sw,
        })
    return in_maps


def run_on_hw(in_maps, trace=False, **kwargs):
    from concourse import bass_utils

    nc = build_nc()
    return bass_utils.run_bass_kernel_spmd(
        nc, in_maps, core_ids=list(range(N_CORES)), trace=trace, **kwargs
    )


def kernel(x, w_attn, w_proj, cos, sin):
    in_maps = host_prep(x, w_attn, w_proj, cos, sin)
    res = run_on_hw(in_maps)
    out = np.zeros((TOK, C), np.float64)
    for r in res.results:
        out += r["out"].astype(np.float64)
    return out.astype(np.float32).reshape(B, T, C)


# revision 13
# speedup vs baseline: 1.0664x; 1.0664x over previous
"""Trainium2 Bass kernel for nn_CausalSelfAttention (GQA + RoPE + qk-RMSNorm).

Strategy (Megatron-style head parallelism over 8 NeuronCores):
  - Each core owns 2 of the 16 q heads and the matching 1 of 8 kv heads.
  - Per core: QKV projection for its 512 rows of w_attn, RoPE + qk RMS norm,
    causal flash-style attention for its (2 q heads x 2 batches), and a
    partial output projection through its 256 columns of w_proj.
  - Host sums the 8 partial outputs (no on-device collectives).

Schedule (v1):
  - All PSUM pools live at root scope (8 banks total) so attention(b0) can
    overlap QKV(b1), and out-proj(b0) can overlap attention(b1).
  - Group 0 runs its QKV matmuls m-inner with per-k DMA chunking so the PE
    starts ~1us into the kernel instead of waiting for the full x0/wq load.
  - RoPE runs per (head-tile, token-group) instead of per batch.
  - Softmax denominator: full k-blocks are pair-summed on GpSimd and
    quad-summed on DVE (bf16), so the PE's ones-matmul only sees ~1/4 of
    the columns; diagonal blocks go straight into the PE accumulation on
    their valid subranges.  Diagonal blocks skip the memset: exp writes the
    valid subrange, the 128-wide triangle gets a mask multiply, and all
    consumers read only the valid subrange.

Matmuls run in bf16 with fp32 PSUM accumulation; softmax statistics fp32.
Self-contained: hardcodes all shapes from the problem spec.
"""

import math
import numpy as np
import ml_dtypes
from contextlib import ExitStack

# ---- problem constants (hardcoded per spec) ----
B, T, C = 2, 2048, 2048
N_HEAD, N_KV_HEAD, HD = 16, 8, 128
KV_DIM = N_KV_HEAD * HD
EPS = 1.1920929e-07
N_CORES = 8
QH_PER_CORE = N_HEAD // N_CORES          # 2
TOK = B * T                              # 4096
P = 128
TG = 512                                 # token group (matmul N)
NT = TOK // TG                           # 8 token groups
KT = C // P                              # 16 contraction tiles
NGB = T // TG                            # 4 q groups per batch
NJB = T // P                             # 16 k tiles per batch
SCALE = 1.0 / math.sqrt(HD)

BF16 = ml_dtypes.bfloat16

_CACHE = {}


# --------------------------------------------------------------------------
# device program
# --------------------------------------------------------------------------

def _emit(tc, out_ap, t_in):
    import concourse.bass as bass  # noqa: F401
    import concourse.mybir as mybir

    f32 = mybir.dt.float32
    bf16 = mybir.dt.bfloat16
    AF = mybir.ActivationFunctionType
    nc = tc.nc

    x_d = t_in["x_sw"]
    wq_d = t_in["wq_sw"]
    wp_d = t_in["wp_sw"]
    cs_d = t_in["cs_sw"]
    mask_d = t_in["mask_sw"]
    eye_d = t_in["eye_sw"]

    with ExitStack() as root:
        # ---------------- PSUM pools (8 banks total, root scope) ----------
        mm_ps = root.enter_context(tc.tile_pool(name="mmps", bufs=2, space="PSUM"))
        ssq_ps = root.enter_context(tc.tile_pool(name="ssqps", bufs=1, space="PSUM"))
        s_ps = root.enter_context(tc.tile_pool(name="sps", bufs=2, space="PSUM"))
        y_ps = root.enter_context(tc.tile_pool(name="yps", bufs=2, space="PSUM"))
        d_ps = root.enter_context(tc.tile_pool(name="dps", bufs=1, space="PSUM"))

        # ---------------- SBUF pools --------------------------------------
        const = root.enter_context(tc.tile_pool(name="const", bufs=1))
        xin = root.enter_context(tc.tile_pool(name="xin", bufs=2))
        big = root.enter_context(tc.tile_pool(name="big", bufs=1))
        sqp = root.enter_context(tc.tile_pool(name="sq", bufs=2))
        sq2p = root.enter_context(tc.tile_pool(name="sq2", bufs=2))
        srp = root.enter_context(tc.tile_pool(name="sr", bufs=2))
        ropet = root.enter_context(tc.tile_pool(name="ropet", bufs=4))
        ptp = root.enter_context(tc.tile_pool(name="pt", bufs=18))
        prp = root.enter_context(tc.tile_pool(name="pr", bufs=6))
        qdp = root.enter_context(tc.tile_pool(name="qd", bufs=3))
        denp = root.enter_context(tc.tile_pool(name="den", bufs=2))
        ostgp = root.enter_context(tc.tile_pool(name="ostg", bufs=3))

        # ---------------- constant loads ----------------------------------
        # group 0 data interleaved per k-tile so the first matmul only waits
        # on ~256KB; supply rate then matches the PE's m-inner consume rate.
        wq_sb = const.tile([P, KT, TG], bf16)
        x0_sb = const.tile([P, KT, TG], bf16, tag="x0")
        for k in range(KT):
            nc.sync.dma_start(out=wq_sb[:, k], in_=wq_d[:, k])
            nc.sync.dma_start(out=x0_sb[:, k], in_=x_d[:, 0, k])
        cs_sb = const.tile([P, 2, T], bf16)
        nc.sync.dma_start(out=cs_sb[:], in_=cs_d)
        mask_sb = const.tile([P, P], bf16)
        nc.sync.dma_start(out=mask_sb[:], in_=mask_d)
        eye_sb = const.tile([P, P], bf16)
        nc.sync.dma_start(out=eye_sb[:], in_=eye_d)
        wp_sb = const.tile([P, QH_PER_CORE, C], bf16)
        nc.sync.dma_start(out=wp_sb[:], in_=wp_d)
        eps_sb = const.tile([P, 1], f32)
        nc.vector.memset(eps_sb[:], EPS)
        onesm_sb = const.tile([P, P], bf16)
        nc.vector.memset(onesm_sb[:], 1.0)

        # post-rope, post-norm q (2 heads) and k, in [d, tok] layout
        qn = [big.tile([P, TOK], bf16, name=f"qn{m}", tag=f"qn{m}") for m in range(3)]
        v_sb = big.tile([P, TOK], bf16, tag="v")
        vT_sb = big.tile([P, 2 * NJB, P], bf16, tag="vT")   # [ktok, (b,j), d]
        yT = [big.tile([P, TOK], bf16, name=f"yT{h}", tag=f"yT{h}") for h in range(QH_PER_CORE)]

        # ------- stage 1 helpers ------------------------------------------
        def qkv_consume(m, n, ps):
            """RMS-norm (m<3) or v-copy (m==3) of one QKV psum tile."""
            nsl = slice(n * TG, (n + 1) * TG)
            if m == 3:
                nc.vector.tensor_copy(v_sb[:, nsl], ps[:])
                return
            # cast to bf16 early (releases the PSUM bank), square on DVE so
            # the ACT engine only ever evaluates {Ln, Exp, Copy} — one table
            # set — and never thrashes table loads in the overlapped schedule.
            sq = sqp.tile([P, TG], bf16)
            nc.vector.tensor_copy(sq[:], ps[:])
            sq2 = sq2p.tile([P, TG], bf16)
            nc.vector.tensor_mul(sq2[:], sq[:], sq[:])
            ssqb = ssq_ps.tile([P, TG], f32)
            nc.tensor.matmul(ssqb[:], onesm_sb[:], sq2[:], start=True, stop=True)
            # rsqrt(ms + eps) = exp(-0.5 * ln(ms + eps))
            srb = srp.tile([P, TG], f32)
            nc.scalar.activation(srb[:], ssqb[:], AF.Ln, bias=eps_sb[:], scale=1.0 / HD)
            nc.scalar.activation(srb[:], srb[:], AF.Exp, scale=-0.5)
            nc.vector.tensor_mul(qn[m][:, nsl], sq[:], srb[:])

        def rope_group(m, n):
            """RoPE for one [128, TG] token-group of head-tile m (in place)."""
            nsl = slice(n * TG, (n + 1) * TG)
            csl = slice((n % NGB) * TG, (n % NGB + 1) * TG)
            t1 = ropet.tile([P, TG], bf16, tag="t1")
            xsw = ropet.tile([P, TG], bf16, tag="xsw")
            nc.gpsimd.dma_start(out=xsw[0:64, :], in_=qn[m][64:128, nsl])
            nc.gpsimd.dma_start(out=xsw[64:128, :], in_=qn[m][0:64, nsl])
            nc.vector.tensor_mul(t1[:], qn[m][:, nsl], cs_sb[:, 0, csl])
            # t1 = [x1*c ; x2*c]; xsw*s2n = [x2*s ; -x1*s]
            nc.vector.tensor_mul(xsw[:], xsw[:], cs_sb[:, 1, csl])
            nc.vector.tensor_add(qn[m][:, nsl], t1[:], xsw[:])

        def v_transpose_group(n):
            # plain matmul against identity: v_blk^T @ I — same result as
            # transpose-mode but runs on the regular (pipelined, HAM-warming)
            # matmul path (~81ns vs ~275ns per 128x128).
            for blk in range(4 * n, 4 * n + 4):
                tp = mm_ps.tile([P, P], f32, tag="mm")
                nc.tensor.matmul(tp[:], v_sb[:, blk * P:(blk + 1) * P], eye_sb[:],
                                 start=True, stop=True)
                nc.vector.tensor_copy(vT_sb[:, blk], tp[:])

        def stage1_group(n, xb):
            for m in range(4):
                ps = mm_ps.tile([P, TG], f32, tag="mm")
                for k in range(KT):
                    nc.tensor.matmul(
                        ps[:], wq_sb[:, k, m * P:(m + 1) * P], xb[:, k],
                        start=(k == 0), stop=(k == KT - 1),
                    )
                qkv_consume(m, n, ps)
                if m < 3:
                    rope_group(m, n)
            v_transpose_group(n)

        # ------- stage 1, group 0: m-inner so the PE starts on k0 ---------
        ps0 = [mm_ps.tile([P, TG], f32, tag="mm", name="ps0a"),
               mm_ps.tile([P, TG], f32, tag="mm", name="ps0b"),
               y_ps.tile([P, TG], f32, tag="y", name="ps0c"),
               d_ps.tile([P, TG], f32, tag="d", name="ps0d")]
        for k in range(KT):
            for m in range(4):
                nc.tensor.matmul(
                    ps0[m][:], wq_sb[:, k, m * P:(m + 1) * P], x0_sb[:, k],
                    start=(k == 0), stop=(k == KT - 1),
                )
        for m in range(4):
            qkv_consume(m, 0, ps0[m])
            if m < 3:
                rope_group(m, 0)
        v_transpose_group(0)

        # ------- stage 1, groups 1..3 (batch 0) ---------------------------
        def stage1_rest(groups):
            for n in groups:
                xb = xin.tile([P, KT, TG], bf16)
                nc.sync.dma_start(out=xb[:, 0:8, :], in_=x_d[:, n, 0:8, :])
                nc.sync.dma_start(out=xb[:, 8:16, :], in_=x_d[:, n, 8:16, :])
                stage1_group(n, xb)

        stage1_rest(range(1, NGB))

        # ------- output projection for a range of 128-token tiles ---------
        def outproj_tiles(tts):
            for tt in tts:
                ost = ostgp.tile([P, C], bf16)
                for og in range(C // TG):
                    op = mm_ps.tile([P, TG], f32, tag="mm")
                    nc.tensor.matmul(
                        op[:], yT[0][:, tt * P:(tt + 1) * P],
                        wp_sb[:, 0, og * TG:(og + 1) * TG],
                        start=True, stop=False,
                    )
                    nc.tensor.matmul(
                        op[:], yT[1][:, tt * P:(tt + 1) * P],
                        wp_sb[:, 1, og * TG:(og + 1) * TG],
                        start=False, stop=True,
                    )
                    if og % 2 == 0:
                        nc.vector.tensor_copy(ost[:, og * TG:(og + 1) * TG], op[:])
                    else:
                        nc.scalar.copy(ost[:, og * TG:(og + 1) * TG], op[:])
                nc.sync.dma_start(out=out_ap[tt * P:(tt + 1) * P, :], in_=ost[:])

        # ------- attention for one batch ----------------------------------
        def attention_batch(b, interleave_outproj=False):
            bT = b * T
            for qh in range(QH_PER_CORE):
                q_t, k_t = qn[qh], qn[2]
                for g in range(NGB):
                    qsl = slice(bT + g * TG, bT + (g + 1) * TG)
                    jmax = 4 * g + 3
                    pts = []
                    for j in range(jmax + 1):
                        off = (j - 4 * g) * P if j >= 4 * g else 0
                        sp = s_ps.tile([P, TG], f32)
                        nc.tensor.matmul(
                            sp[:, off:],
                            k_t[:, bT + j * P: bT + (j + 1) * P],
                            q_t[:, qsl][:, off:],
                            start=True, stop=True,
                        )
                        pt = ptp.tile([P, TG], bf16)
                        nc.scalar.activation(pt[:, off:], sp[:, off:], AF.Exp,
                                             scale=SCALE)
                        if j >= 4 * g:
                            # triangle mask on the diagonal 128 columns
                            nc.vector.tensor_mul(
                                pt[:, off:off + P], pt[:, off:off + P],
                                mask_sb[:, 0:P],
                            )
                        pts.append((pt, off))
                    # AV accumulation (diagonal blocks on their subranges)
                    yp = y_ps.tile([P, TG], f32, tag="y")
                    for idx, (pt, off) in enumerate(pts):
                        nc.tensor.matmul(
                            yp[:, off:], vT_sb[:, b * NJB + idx], pt[:, off:],
                            start=(idx == 0), stop=(idx == jmax),
                        )
                    # denominator: pair (gpsimd) + quad (DVE) tree over the
                    # full blocks, then a short ones-matmul accumulation.
                    nfull = 4 * g
                    pairs = []
                    for i in range(nfull // 2):
                        pr = prp.tile([P, TG], bf16)
                        nc.gpsimd.tensor_add(pr[:], pts[2 * i][0][:],
                                             pts[2 * i + 1][0][:])
                        pairs.append(pr)
                    quads = []
                    for i in range(nfull // 4):
                        qd = qdp.tile([P, TG], bf16)
                        nc.vector.tensor_add(qd[:], pairs[2 * i][:],
                                             pairs[2 * i + 1][:])
                        quads.append(qd)
                    terms = [(qd[:], 0) for qd in quads] + \
                            [(pt[:], off) for (pt, off) in pts[nfull:]]
                    dp = d_ps.tile([P, TG], f32, tag="d")
                    last = len(terms) - 1
                    for idx, (tsb, off) in enumerate(terms):
                        nc.tensor.matmul(
                            dp[:, off:], onesm_sb[:], tsb[:, off:],
                            start=(idx == 0), stop=(idx == last),
                        )
                    den = denp.tile([P, TG], f32)
                    nc.vector.reciprocal_approx_fast(den[:], dp[:])
                    nc.vector.tensor_mul(yT[qh][:, qsl], yp[:], den[:])
                    if interleave_outproj and qh == QH_PER_CORE - 1:
                        # this token range now has both heads' yT: project it
                        outproj_tiles(range(b * NJB + 4 * g, b * NJB + 4 * g + 4))

        # high priority: attention(b0); backfill: stage1(b1); then
        # attention(b1) with its out-proj interleaved per token-group;
        # out-proj(b0) last (lowest priority — pure PE backfill).
        attention_batch(0)
        stage1_rest(range(NGB, NT))
        attention_batch(1, interleave_outproj=True)
        outproj_tiles(range(0, NJB))


def _prefer_combined_act_table():
    """Reorder the ACT table-set list so `natural_log_exp_and_others` (which
    contains every function this kernel uses: Exp, Ln, Square, Copy,
    Identity) is considered first.  Without this, the table-load pass maps
    Exp -> exp_and_others and Ln -> natural_log, and the overlapped schedule
    ping-pongs ~48 ACT_TABLE_LOADs (~62us of ScalarE time)."""
    import concourse.bacc as bacc_mod
    import concourse.hw_specs as hw_specs

    if getattr(bacc_mod, "_combined_act_tables", False):
        return
    orig = hw_specs.get_activation_tables

    def combined_only(arch):
        # Keep list order (act_func_set_id is the index into the canonical
        # act_info.json list!) but empty every set except the combined one,
        # so the chooser must place all activations there.
        tabs = orig(arch)
        pref = "natural_log_exp_and_others"
        if pref not in tabs:
            return tabs
        return {k: (v if k == pref else set()) for k, v in tabs.items()}

    bacc_mod.get_activation_tables = combined_only
    bacc_mod._combined_act_tables = True


def build_nc():
    """Build and compile the (single, shared across cores) Bass program."""
    if "nc" in _CACHE:
        return _CACHE["nc"]
    import concourse.mybir as mybir
    import concourse.tile as tile
    from concourse import bacc

    _prefer_combined_act_table()

    f32 = mybir.dt.float32  # noqa: F841
    bf16 = mybir.dt.bfloat16

    nc = bacc.Bacc("TRN2", target_bir_lowering=False, debug=False)
    shapes = {
        "x_sw": ((P, NT, KT, TG), bf16),
        "wq_sw": ((P, KT, TG), bf16),
        "wp_sw": ((P, QH_PER_CORE, C), bf16),
        "cs_sw": ((P, 2, T), bf16),
        "mask_sw": ((P, P), bf16),
        "eye_sw": ((P, P), bf16),
    }
    t_in = {
        name: nc.dram_tensor(name, shape, dt, kind="ExternalInput").ap()
        for name, (shape, dt) in shapes.items()
    }
    out_ap = nc.dram_tensor("out", (TOK, C), bf16, kind="ExternalOutput").ap()

    with tile.TileContext(nc) as tc:
        _emit(tc, out_ap, t_in)
    nc.compile()
    _CACHE["nc"] = nc
    return nc


# --------------------------------------------------------------------------
# host-side data preparation
# --------------------------------------------------------------------------

def _swizzle_ktiles(a2d):
    """[R*128, F] -> [128, R, F] picking partition-within-tile as leading."""
    r128, f = a2d.shape
    r = r128 // P
    return np.ascontiguousarray(a2d.reshape(r, P, f).transpose(1, 0, 2))


def host_prep(x, w_attn, w_proj, cos, sin):
    x = np.asarray(x, np.float32)
    w_attn = np.asarray(w_attn, np.float32)
    w_proj = np.asarray(w_proj, np.float32)
    cos = np.asarray(cos, np.float32).reshape(T, HD // 2)
    sin = np.asarray(sin, np.float32).reshape(T, HD // 2)

    # x: (B,T,C) -> xT (C, TOK) -> [128, n, k, t]
    xT = x.reshape(TOK, C).T                        # (C, TOK)
    x_sw = (
        xT.reshape(KT, P, NT, TG).transpose(1, 2, 0, 3)  # (P, n, k, t)
    )
    x_sw = np.ascontiguousarray(x_sw).astype(BF16)

    # cos/sin duplicated across both 64-partition halves: [128, 2, T]
    c2 = np.concatenate([cos.T, cos.T], axis=0)     # (128, T)
    s2 = np.concatenate([sin.T, -sin.T], axis=0)    # sign-folded for rope add
    cs_sw = np.stack([c2, s2], axis=1).astype(BF16)  # (128, 2, T)

    # causal triangle mask for the diagonal 128 columns: keep col >= row
    col = np.arange(P)[None, :]
    row = np.arange(P)[:, None]
    mask_sw = (col >= row).astype(BF16)              # (128, 128)

    eye_sw = np.eye(P, dtype=np.float32).astype(BF16)

    in_maps = []
    for c in range(N_CORES):
        qrows = w_attn[QH_PER_CORE * HD * c: QH_PER_CORE * HD * (c + 1)]
        krows = w_attn[C + HD * c: C + HD * (c + 1)]
        vrows = w_attn[C + KV_DIM + HD * c: C + KV_DIM + HD * (c + 1)]
        w_sel = np.concatenate([qrows, krows, vrows], axis=0)   # (512, C)
        wq_sw = _swizzle_ktiles(w_sel.T).astype(BF16)           # (128, 16, 512)

        wp_sel = w_proj[:, QH_PER_CORE * HD * c: QH_PER_CORE * HD * (c + 1)]
        wp_sw = _swizzle_ktiles(np.ascontiguousarray(wp_sel.T)).astype(BF16)

        in_maps.append({
            "x_sw": x_sw,
            "wq_sw": np.ascontiguousarray(wq_sw.reshape(P, KT, TG)),
            "wp_sw": np.ascontiguousarray(wp_sw.reshape(P, QH_PER_CORE, C)),
            "cs_sw": cs_sw,
            "mask_sw": mask_sw,
            "eye_sw": eye_sw,
        })
    return in_maps


def run_on_hw(in_maps, trace=False, **kwargs):
    from concourse import bass_utils

    nc = build_nc()
    return bass_utils.run_bass_kernel_spmd(
        nc, in_maps, core_ids=list(range(N_CORES)), trace=trace, **kwargs
    )


def kernel(x, w_attn, w_proj, cos, sin):
    in_maps = host_prep(x, w_attn, w_proj, cos, sin)
    res = run_on_hw(in_maps)
    out = np.zeros((TOK, C), np.float64)
    for r in res.results:
        out += r["out"].astype(np.float64)
    return out.astype(np.float32).reshape(B, T, C)
